# revision 1
# baseline (speedup 1.0000x reference)
"""Trainium2 Bass kernel for nn_MemConLoss_trans (supervised-contrastive loss
with memory-bank hard negatives).

Strategy (8 NeuronCores, SPMD):
  - mem_bank sharded along M (8192 rows/core); s_box_feat / s_query sharded
    along B (128 rows/core); mem_s_query replicated.
  - Each core: spatial-mean of its box shard -> nq shard (fp16), device
    AllGather -> full negated query matrix; DMA-transposes build [D, *]
    operand layouts; fp16 matmul streams -score = nq @ bank_shard.T through
    PSUM; PSUM chunks are evacuated to fp16 and reduced with a slot-max
    (elementwise max over chunks) + max8 to the per-row top-8 largest
    -score (= 8 smallest raw scores) of the shard.
  - The small [B,B] contrastive logits are data-parallel over B: each core
    l2-normalizes, computes its 128xB logit rows via fp32r matmul and
    row-sums exp(l - 4.0) on the scalar engine.
  - Host merges: top-5 smallest scores from 64 candidates/row, final
    log/mean reduction in fp64.

The constant shift 4.0 stands in for the per-row logits max: the reference's
row max only enters through exp(max)*sum(exp(neg)) ~ 1e-6 of each row's
total, so a constant within ~1 of the true max changes the loss by < 1e-5
relative.
"""

import numpy as np

B = 1024
D = 256
HWSP = 49          # 7*7 spatial positions
NCORES = 8
BD = B // NCORES   # 128 rows of B per core
MC = 65536 // NCORES  # 8192 rows of mem_bank per core
NBT = B // 128     # 8 b-tiles of the score matmul per core
MX = 4.0           # constant stand-in for the per-row logits max
TEMP = 0.07

_CACHE = {}


def _build_module():
    import os
    import concourse.bacc as bacc
    import concourse.mybir as mybir
    import concourse.tile as tile

    bisect = os.environ.get("KBISECT", "")

    F32 = mybir.dt.float32
    F32R = mybir.dt.float32r
    F16 = mybir.dt.float16
    AF = mybir.ActivationFunctionType
    ALU = mybir.AluOpType
    X = mybir.AxisListType.X

    nc = bacc.Bacc("TRN2", target_bir_lowering=False, debug=False,
                   enable_asserts=False, num_devices=NCORES)

    box = nc.dram_tensor("box", [BD, D * HWSP], F32, kind="ExternalInput").ap()
    sq = nc.dram_tensor("sq", [BD, D], F32, kind="ExternalInput").ap()
    msq = nc.dram_tensor("msq", [B, D], F32, kind="ExternalInput").ap()
    bank = nc.dram_tensor("bank", [MC, D], F32, kind="ExternalInput").ap()
    ident = nc.dram_tensor("ident", [128, 128], F32, kind="ExternalInput").ap()
    o_top8 = nc.dram_tensor("o_top8", [B, 8], F16, kind="ExternalOutput").ap()
    o_rowsum = nc.dram_tensor("o_rowsum", [BD, 1], F32, kind="ExternalOutput").ap()

    with tile.TileContext(nc) as tc:
        with (
            tc.tile_pool(name="big", bufs=1) as big,
            tc.tile_pool(name="stage", bufs=4) as stage,
            tc.tile_pool(name="small", bufs=2) as small,
            tc.tile_pool(name="evac", bufs=3) as evacp,
            tc.tile_pool(name="runp", bufs=2) as runp,
            tc.tile_pool(name="dram", bufs=1, space="DRAM") as dram,
        ):
            # ---------------- phase Q: box spatial mean -> nq, AllGather ---
            box_sb = big.tile([BD, D * HWSP], F32)
            qsum = small.tile([BD, D], F32)
            for k in range(8):
                w = D * HWSP // 8  # 1568 = 32 d-slots * 49
                nc.sync.dma_start(box_sb[:, k * w:(k + 1) * w],
                                  box[:, k * w:(k + 1) * w])
                nc.vector.tensor_reduce(
                    qsum[:, k * 32:(k + 1) * 32],
                    box_sb[:, k * w:(k + 1) * w].rearrange(
                        "p (d h) -> p d h", h=HWSP),
                    axis=X, op=ALU.add)
            nq16 = small.tile([BD, D], F16)
            nc.vector.tensor_scalar(out=nq16[:], in0=qsum[:],
                                    scalar1=-1.0 / HWSP, scalar2=None,
                                    op0=ALU.mult)
            ag_in = dram.tile([BD, D], F16)
            ag_out = dram.tile([B, D], F16)
            nc.sync.dma_start(ag_in[:], nq16[:])
            nc.gpsimd.collective_compute(
                "AllGather", ALU.bypass,
                replica_groups=[list(range(NCORES))],
                ins=[ag_in.opt()], outs=[ag_out.opt()],
            )
            nqT = [big.tile([128, B], F16, name=f"nqT{c}") for c in range(2)]

            # ---------------- phase LOGITS loads (early, small) ------------
            ident_sb = small.tile([128, 128], F32)
            nc.sync.dma_start(ident_sb[:], ident)
            bias_mx = small.tile([128, 1], F32)
            nc.vector.memset(bias_mx[:], -MX)

            at = small.tile([BD, D], F32)
            nc.sync.dma_start(at[:], sq)
            cts = [stage.tile([128, D], F32, name=f"ct{j}") for j in range(8)]
            for j in range(8):
                nc.sync.dma_start(cts[j][:], msq[j * 128:(j + 1) * 128, :])

            # ---------------- phase BANK: cast to DRAM f16, transpose-load -
            # gate: holds the in-order gpsimd stream (and so the bank cast
            # DMA traffic) until the box loads have landed, keeping HBM
            # bandwidth free for the AllGather critical path.
            gate_t = small.tile([128, 8], F32)
            nc.gpsimd.tensor_copy(gate_t[:], box_sb[:, D * HWSP - 8:])
            bank_f16d = dram.tile([MC, D], F16)
            for k in range(16):
                rows = MC // 16  # 512
                nc.gpsimd.dma_start(bank_f16d[k * rows:(k + 1) * rows, :],
                                    bank[k * rows:(k + 1) * rows, :])
            bankT = [big.tile([128, MC], F16, name=f"bankT{c}") for c in range(2)]
            for t in range(4):
                for c in range(2):
                    rows = MC // 4  # 2048
                    nc.sync.dma_start(
                        bankT[c][:, t * rows:(t + 1) * rows],
                        bank_f16d[t * rows:(t + 1) * rows,
                                  c * 128:(c + 1) * 128],
                        transpose=True)
            # nqT[c]: [128 d, 1024 b] fp16 (after bankT so the waiting
            # transposes don't stall the in-order sync stream)
            for c in range(2):
                nc.sync.dma_start(nqT[c][:], ag_out[:, c * 128:(c + 1) * 128],
                                  transpose=True)

            # ---------------- phase LOGITS compute -------------------------

            scr = small.tile([128, D], F32)
            for idx, t in enumerate([at] + cts):
                ss = small.tile([128, 1], F32, name=f"ss{idx}", tag="ss")
                nc.scalar.activation(scr[:], t[:], AF.Square, accum_out=ss[:])
                nc.scalar.activation(ss[:], ss[:], AF.Sqrt)
                nc.vector.tensor_scalar(out=ss[:], in0=ss[:], scalar1=1e-12,
                                        scalar2=None, op0=ALU.max)
                rinv = small.tile([128, 1], F32, name=f"rinv{idx}", tag="rinv")
                nc.vector.reciprocal(rinv[:], ss[:])
                if idx == 0:  # anchor also carries 1/TEMP
                    nc.vector.tensor_scalar(out=rinv[:], in0=rinv[:],
                                            scalar1=1.0 / TEMP, scalar2=None,
                                            op0=ALU.mult)
                nc.vector.tensor_scalar(out=t[:], in0=t[:],
                                        scalar1=rinv[:, 0:1], scalar2=None,
                                        op0=ALU.mult)

            atT = [small.tile([128, 128], F32, name=f"atT{c}") for c in range(2)]
            ctT = [big.tile([128, B], F32, name=f"ctT{c}") for c in range(2)]
            with tc.tile_pool(name="psT", bufs=2, space="PSUM") as psT:
                for c in range(2):
                    pt = psT.tile([128, 128], F32, tag="pt")
                    nc.tensor.transpose(pt[:], at[:, c * 128:(c + 1) * 128],
                                        ident_sb[:])
                    nc.vector.tensor_copy(atT[c][:], pt[:])
                for j in range(8):
                    for c in range(2):
                        pt = psT.tile([128, 128], F32, tag="pt")
                        nc.tensor.transpose(pt[:],
                                            cts[j][:, c * 128:(c + 1) * 128],
                                            ident_sb[:])
                        nc.vector.tensor_copy(ctT[c][:, j * 128:(j + 1) * 128],
                                              pt[:])

            with tc.tile_pool(name="psL", bufs=1, space="PSUM") as psL:
                pl = psL.tile([128, B], F32)
                for jc in range(2):
                    for c in range(2):
                        nc.tensor.matmul(
                            pl[:, jc * 512:(jc + 1) * 512],
                            atT[c][:],
                            ctT[c][:, jc * 512:(jc + 1) * 512],
                            start=(c == 0), stop=(c == 1))
                rs = small.tile([128, 1], F32)
                nc.scalar.activation(pl[:], pl[:], AF.Exp, bias=bias_mx[:, 0:1],
                                     accum_out=rs[:])
                nc.sync.dma_start(o_rowsum, rs[:])

            # ---------------- phase SCORE: -score matmul + topk ------------
            if "noscore" in bisect:
                zt8 = small.tile([128, 8], F16, tag="t8")
                nc.vector.memset(zt8[:], -20.0)
                for bt in range(NBT):
                    nc.sync.dma_start(o_top8[bt * 128:(bt + 1) * 128, :], zt8[:])
            elif True:
              with tc.tile_pool(name="psS", bufs=2, space="PSUM") as psS:
                  for bt in range(NBT):
                      run = runp.tile([128, 512], F16, tag="run")
                      for q4 in range(4):
                          ps = psS.tile([128, 2048], F32, tag="ps")
                          for k in range(4):
                              m0 = (q4 * 4 + k) * 512
                              for c in range(2):
                                  nc.tensor.matmul(
                                      ps[:, k * 512:(k + 1) * 512],
                                      nqT[c][:, bt * 128:(bt + 1) * 128],
                                      bankT[c][:, m0:m0 + 512],
                                      start=(c == 0), stop=(c == 1))
                          if q4 == 3 and (bt % 2 == 1):
                              # DVE-direct slot-max from PSUM (load balance)
                              for k in range(4):
                                  nc.vector.tensor_tensor(
                                      out=run[:], in0=ps[:, k * 512:(k + 1) * 512],
                                      in1=run[:], op=ALU.max)
                          else:
                              ev = evacp.tile([128, 2048], F16, tag="ev")
                              nc.scalar.activation(ev[:], ps[:], AF.Copy)
                              k0 = 0
                              if q4 == 0:
                                  nc.vector.tensor_copy(run[:], ev[:, 0:512])
                                  k0 = 1
                              for k in range(k0, 4):
                                  nc.vector.tensor_tensor(
                                      out=run[:], in0=ev[:, k * 512:(k + 1) * 512],
                                      in1=run[:], op=ALU.max)
                      t8 = small.tile([128, 8], F16, tag="t8")
                      nc.vector.max(t8[:], run[:])
                      nc.sync.dma_start(o_top8[bt * 128:(bt + 1) * 128, :], t8[:])

    nc.compile()
    return nc


def _get_module():
    if "nc" not in _CACHE:
        _CACHE["nc"] = _build_module()
    return _CACHE["nc"]


def _make_in_maps(inputs):
    box = np.ascontiguousarray(inputs["s_box_feat"], dtype=np.float32)
    box = box.reshape(B, D * HWSP)
    sq = np.ascontiguousarray(inputs["s_query"], dtype=np.float32)
    msq = np.ascontiguousarray(inputs["mem_s_query"], dtype=np.float32)
    bank = np.ascontiguousarray(inputs["mem_bank"], dtype=np.float32)
    eye = np.eye(128, dtype=np.float32)
    in_maps = []
    for c in range(NCORES):
        in_maps.append({
            "box": np.ascontiguousarray(box[c * BD:(c + 1) * BD]),
            "sq": np.ascontiguousarray(sq[c * BD:(c + 1) * BD]),
            "msq": msq,
            "bank": np.ascontiguousarray(bank[c * MC:(c + 1) * MC]),
            "ident": eye,
        })
    return in_maps


def _finalize(inputs, results):
    # results: list (per core) of dict name -> np.ndarray
    cand = np.concatenate(
        [np.asarray(r["o_top8"], dtype=np.float32) for r in results], axis=1)
    rowsum = np.concatenate(
        [np.asarray(r["o_rowsum"], dtype=np.float64)[:, 0] for r in results])

    # 5 smallest raw scores per row = 5 largest of the gathered -score cands
    top5 = -np.sort(-cand, axis=1)[:, :5]
    neg = (-top5).astype(np.float64)
    negsum = np.exp(neg).sum(axis=1)

    # host-side diagonal of the contrastive logits (fp32, mirrors reference)
    a = np.asarray(inputs["s_query"], dtype=np.float32)
    cf = np.asarray(inputs["mem_s_query"], dtype=np.float32)
    an = a / np.maximum(np.linalg.norm(a, axis=1, keepdims=True), 1e-12)
    cn = cf / np.maximum(np.linalg.norm(cf, axis=1, keepdims=True), 1e-12)
    diag = (np.einsum("ij,ij->i", an.astype(np.float32),
                      cn.astype(np.float32)).astype(np.float32)
            / np.float32(TEMP)).astype(np.float64)

    loss_i = np.log(rowsum + np.exp(-MX) * negsum) - (diag - MX)
    m = loss_i.mean()
    if np.isnan(m):
        m = 0.0
    return np.float32(m)


def run(inputs, trace=False, **spmd_kwargs):
    from concourse.bass_utils import run_bass_kernel_spmd
    nc = _get_module()
    in_maps = _make_in_maps(inputs)
    res = run_bass_kernel_spmd(nc, in_maps, core_ids=list(range(NCORES)),
                               trace=trace, **spmd_kwargs)
    loss = _finalize(inputs, res.results)
    return loss, res


def kernel(**inputs) -> np.ndarray:
    loss, _ = run(inputs, trace=False)
    return loss



# revision 5
# speedup vs baseline: 1.3624x; 1.3624x over previous
"""Trainium2 Bass kernel v2 for nn_MemConLoss_trans.

Redesign vs baseline:
  - Bank path: bulk fp32 loads -> PE transposes (fp32, 2cyc/row) -> evac-cast
    to fp8e4 on Act/DVE/Pool. No DRAM f16 round-trip, no XBAR DMA transpose.
  - Score matmul: fp8e4 DoubleRow (K=256 packed as 2 planes of 128) at
    0.5 cyc/row = 2x fp16 throughput.
  - AllGather payload: transposed fp8 nq [256 x 128] = 32 KB/core (vs 64 KB
    f16), gathered blocks bulk-loaded into the plane-major lhsT stack.
  - Score reduce: slot-max over 16 chunks/bt split across Act-evac+DVE-f16max,
    DVE-direct-PSUM, Pool-direct-PSUM; then max8.
  - Logits: data-parallel over B as baseline, but f16 matmul operands and
    batched PSUM-transpose evacs.

Scale: nq is scaled by -S/49 (S=8) before fp8 cast; PSUM holds -S*score.
Host merges per-core top-8 candidates and divides by S.
"""

import numpy as np

B = 1024
D = 256
HWSP = 49
NCORES = 8
BD = B // NCORES      # 128
MC = 65536 // NCORES  # 8192 bank rows per core
S = 8.0               # fp8 query scale
MX = 4.0              # constant stand-in for per-row logits max
TEMP = 0.07

_CACHE = {}


def _build_module():
    import concourse.bacc as bacc
    import concourse.mybir as mybir
    import concourse.tile as tile

    F32 = mybir.dt.float32
    F16 = mybir.dt.float16
    F8 = mybir.dt.float8e4
    AF = mybir.ActivationFunctionType
    ALU = mybir.AluOpType
    X = mybir.AxisListType.X
    DR = mybir.MatmulPerfMode.DoubleRow

    nc = bacc.Bacc("TRN2", target_bir_lowering=False, debug=False,
                   enable_asserts=False, num_devices=NCORES)

    box = nc.dram_tensor("box", [BD, D * HWSP], F32, kind="ExternalInput").ap()
    sq = nc.dram_tensor("sq", [BD, D], F32, kind="ExternalInput").ap()
    msq = nc.dram_tensor("msq", [B, D], F32, kind="ExternalInput").ap()
    bank = nc.dram_tensor("bank", [MC, D], F32, kind="ExternalInput").ap()
    ident = nc.dram_tensor("ident", [128, 128], F32, kind="ExternalInput").ap()
    o_cand = nc.dram_tensor("o_cand", [B, 8], F16, kind="ExternalOutput").ap()
    o_rowsum = nc.dram_tensor("o_rowsum", [BD, 1], F32, kind="ExternalOutput").ap()

    NBT = B // 128          # 8 b-tiles
    NMK = MC // 512         # 16 m-chunks of 512
    NBG = MC // 512         # 16 bank dma groups (512 rows each)

    with tile.TileContext(nc) as tc:
        with (
            tc.tile_pool(name="boxp", bufs=3) as boxp,
            tc.tile_pool(name="qp", bufs=1) as qp,
            tc.tile_pool(name="bkg", bufs=4) as bkgp,
            tc.tile_pool(name="big", bufs=1) as big,
            tc.tile_pool(name="lg", bufs=1) as lgp,
            tc.tile_pool(name="evp", bufs=3) as evp,
            tc.tile_pool(name="runp", bufs=2) as runp,
            tc.tile_pool(name="small", bufs=2) as small,
            tc.tile_pool(name="dram", bufs=1, space="DRAM") as dram,
        ):
            # ---------- phase Q: box -> qsum (DVE/Pool split) ----------
            qsum = qp.tile([BD, D], F32)
            NBC = 8
            w = D * HWSP // NBC   # 1568 = 32 d-slots * 49
            for k in range(NBC):
                bch = boxp.tile([BD, w], F32, tag="bch")
                nc.sync.dma_start(bch[:], box[:, k * w:(k + 1) * w])
                nc.vector.tensor_reduce(
                    qsum[:, k * 32:(k + 1) * 32],
                    bch[:].rearrange("p (d h) -> p d h", h=HWSP),
                    axis=X, op=ALU.add)

            ident_sb = small.tile([128, 128], F32)
            nc.sync.dma_start(ident_sb[:], ident)

            # ---------- phase NQ: transpose + fp8 cast + AllGather ----------
            cc_in = dram.tile([128, 256], F8)
            ag_out = dram.tile([B, 256], F8)
            nqStack = big.tile([128, 2 * B], F8)   # free = (core, half, b)

            with tc.tile_pool(name="psQ", bufs=2, space="PSUM") as psQ:
                ptq = psQ.tile([128, 256], F32, tag="ptq")
                for h in range(2):
                    nc.tensor.transpose(ptq[:, h * 128:(h + 1) * 128],
                                        qsum[:, h * 128:(h + 1) * 128],
                                        ident_sb[:])
                sb_cc = small.tile([128, 256], F8)
                nc.scalar.activation(sb_cc[:], ptq[:], AF.Copy,
                                     scale=-S / HWSP)
                nc.sync.dma_start(cc_in[:], sb_cc[:])
                nc.gpsimd.collective_compute(
                    "AllGather", ALU.bypass,
                    replica_groups=[list(range(NCORES))],
                    ins=[cc_in.opt()], outs=[ag_out.opt()],
                )
                for r in range(NCORES):
                    nc.sync.dma_start(nqStack[:, r * 256:(r + 1) * 256],
                                      ag_out[r * 128:(r + 1) * 128, :])

                # ---------- logits input loads + normalize (f32) --------
                at = lgp.tile([128, D], F32, name="at")
                nc.scalar.dma_start(at[:], sq)
                cts = [lgp.tile([128, D], F32, name=f"ct{j}") for j in range(8)]
                for j in range(8):
                    nc.scalar.dma_start(cts[j][:], msq[j * 128:(j + 1) * 128, :])

                scr = small.tile([128, D], F32)
                for idx, t in enumerate([at] + cts):
                    ss = small.tile([128, 1], F32, name=f"ss{idx}", tag="ss")
                    nc.scalar.activation(scr[:], t[:], AF.Square, accum_out=ss[:])
                    nc.scalar.activation(ss[:], ss[:], AF.Sqrt)
                    nc.vector.tensor_scalar(out=ss[:], in0=ss[:], scalar1=1e-12,
                                            scalar2=None, op0=ALU.max)
                    rinv = small.tile([128, 1], F32, name=f"rinv{idx}", tag="rinv")
                    nc.vector.reciprocal(rinv[:], ss[:])
                    if idx == 0:
                        nc.vector.tensor_scalar(out=rinv[:], in0=rinv[:],
                                                scalar1=1.0 / TEMP, scalar2=None,
                                                op0=ALU.mult)
                    nc.vector.tensor_scalar(out=t[:], in0=t[:],
                                            scalar1=rinv[:, 0:1], scalar2=None,
                                            op0=ALU.mult)

                # logits transposes -> f16 atT/ctT (plane-major halves)
                atT = lgp.tile([128, 256], F16, name="atT")    # (h, i)
                ctT = lgp.tile([128, 2 * B], F16, name="ctT")  # (h, j)
                for h in range(2):
                    pt = psQ.tile([128, 256], F32, tag="ptq")
                    nc.tensor.transpose(pt[:, 0:128],
                                        at[:, h * 128:(h + 1) * 128], ident_sb[:])
                    nc.scalar.activation(atT[:, h * 128:(h + 1) * 128],
                                         pt[:, 0:128], AF.Copy)
                for j in range(8):
                    for h in range(2):
                        pt = psQ.tile([128, 256], F32, tag="ptq")
                        nc.tensor.transpose(pt[:, 0:128],
                                            cts[j][:, h * 128:(h + 1) * 128],
                                            ident_sb[:])
                        dst = ctT[:, h * B + j * 128:h * B + (j + 1) * 128]
                        if (j + h) % 2 == 0:
                            nc.scalar.activation(dst, pt[:, 0:128], AF.Copy)
                        else:
                            nc.vector.tensor_copy(dst, pt[:, 0:128])

            # ---------- phase BANK: bulk load + PE transpose + fp8 evac ----
            bankT = big.tile([128, 2 * MC], F8)   # free = (half, m)
            with tc.tile_pool(name="psT", bufs=4, space="PSUM") as psT:
                for g in range(NBG):
                    bkg = bkgp.tile([128, 4 * D], F32, tag="bkg")
                    nc.sync.dma_start(
                        bkg[:],
                        bank[g * 512:(g + 1) * 512, :].rearrange(
                            "(i p) d -> p i d", i=4))
                    for h in range(2):
                        pt = psT.tile([128, 512], F32, tag="pt")
                        for i in range(4):
                            nc.tensor.transpose(
                                pt[:, i * 128:(i + 1) * 128],
                                bkg[:, i * D + h * 128:i * D + h * 128 + 128],
                                ident_sb[:])
                        dst = bankT[:, h * MC + g * 512:h * MC + (g + 1) * 512]
                        r = (2 * g + h) % 8
                        if r in (0, 1, 2):
                            nc.scalar.activation(dst, pt[:], AF.Copy)
                        else:
                            nc.vector.tensor_copy(dst, pt[:])

            # ---------- phase SCORE: fp8 DoubleRow matmul + topk ----------
            # lhsT: nqStack[(bt), (h, b)] ; rhs: bankT[(h, m)]
            bias_mx = small.tile([128, 1], F32)
            nc.vector.memset(bias_mx[:], -MX)

            # reduce rotation per super-chunk (8x [128,1024] per bt):
            # 'a' = Act evac to f16 + DVE f16 maxes (chain A);
            # 'd' = DVE direct-PSUM maxes (chain B). Pool has no max op.
            ROT = ['a', 'd', 'a', 'a', 'd', 'a', 'a', 'a']

            with (
                tc.tile_pool(name="psS", bufs=3, space="PSUM") as psS,
                tc.tile_pool(name="psL", bufs=1, space="PSUM") as psL,
            ):
                rhs_all = bankT[:].rearrange("p (h m) -> p h m", h=2)
                for bt in range(NBT):
                    runA = runp.tile([128, 512], F16, tag="runA")
                    runB = runp.tile([128, 512], F16, tag="runB")
                    nc.vector.memset(runA[:], -60000.0)
                    nc.vector.memset(runB[:], -60000.0)
                    lhs = nqStack[:, bt * 256:(bt + 1) * 256].rearrange(
                        "p (h b) -> p h b", h=2)
                    for q in range(8):      # super-chunks of 1024 m
                        ps = psS.tile([128, 1024], F32, tag="ps")
                        for half in range(2):
                            k = 2 * q + half
                            nc.tensor.matmul(
                                ps[:, half * 512:(half + 1) * 512],
                                lhs, rhs_all[:, :, k * 512:(k + 1) * 512],
                                start=True, stop=True, perf_mode=DR)
                        if ROT[q] == 'a':
                            ev = evp.tile([128, 1024], F16, tag="ev")
                            nc.scalar.activation(ev[:], ps[:], AF.Copy)
                            for half in range(2):
                                nc.vector.tensor_tensor(
                                    out=runA[:],
                                    in0=ev[:, half * 512:(half + 1) * 512],
                                    in1=runA[:], op=ALU.max)
                        else:
                            for half in range(2):
                                nc.vector.tensor_tensor(
                                    out=runB[:],
                                    in0=ps[:, half * 512:(half + 1) * 512],
                                    in1=runB[:], op=ALU.max)
                    nc.vector.tensor_tensor(out=runA[:], in0=runB[:],
                                            in1=runA[:], op=ALU.max)
                    t8 = small.tile([128, 8], F16, tag="t8")
                    nc.vector.max(t8[:], runA[:])
                    nc.sync.dma_start(o_cand[bt * 128:(bt + 1) * 128, :], t8[:])

                # ---------- phase LOGITS matmul + exp rowsum ----------
                pl = psL.tile([128, B], F32)
                for jc in range(2):
                    for h in range(2):
                        nc.tensor.matmul(
                            pl[:, jc * 512:(jc + 1) * 512],
                            atT[:, h * 128:(h + 1) * 128],
                            ctT[:, h * B + jc * 512:h * B + (jc + 1) * 512],
                            start=(h == 0), stop=(h == 1))
                rs = small.tile([128, 1], F32)
                nc.scalar.activation(pl[:], pl[:], AF.Exp, bias=bias_mx[:, 0:1],
                                     accum_out=rs[:])
                nc.sync.dma_start(o_rowsum, rs[:])

    nc.compile()
    return nc


def _get_module():
    if "nc" not in _CACHE:
        _CACHE["nc"] = _build_module()
    return _CACHE["nc"]


def _make_in_maps(inputs):
    box = np.ascontiguousarray(inputs["s_box_feat"], dtype=np.float32)
    box = box.reshape(B, D * HWSP)
    sq = np.ascontiguousarray(inputs["s_query"], dtype=np.float32)
    msq = np.ascontiguousarray(inputs["mem_s_query"], dtype=np.float32)
    bank = np.ascontiguousarray(inputs["mem_bank"], dtype=np.float32)
    eye = np.eye(128, dtype=np.float32)
    in_maps = []
    for c in range(NCORES):
        in_maps.append({
            "box": np.ascontiguousarray(box[c * BD:(c + 1) * BD]),
            "sq": np.ascontiguousarray(sq[c * BD:(c + 1) * BD]),
            "msq": msq,
            "bank": np.ascontiguousarray(bank[c * MC:(c + 1) * MC]),
            "ident": eye,
        })
    return in_maps


def _finalize(inputs, results):
    # cand holds top-8 largest of (-S * score) per row per core
    cand = np.concatenate(
        [np.asarray(r["o_cand"], dtype=np.float32) for r in results], axis=1)
    rowsum = np.concatenate(
        [np.asarray(r["o_rowsum"], dtype=np.float64)[:, 0] for r in results])

    top5 = np.sort(cand, axis=1)[:, -5:]          # 5 largest of -S*score
    neg = (-top5 / S).astype(np.float64)          # 5 smallest raw scores
    negsum = np.exp(neg).sum(axis=1)

    a = np.asarray(inputs["s_query"], dtype=np.float32)
    cf = np.asarray(inputs["mem_s_query"], dtype=np.float32)
    an = a / np.maximum(np.linalg.norm(a, axis=1, keepdims=True), 1e-12)
    cn = cf / np.maximum(np.linalg.norm(cf, axis=1, keepdims=True), 1e-12)
    diag = (np.einsum("ij,ij->i", an, cn).astype(np.float32)
            / np.float32(TEMP)).astype(np.float64)

    loss_i = np.log(rowsum + np.exp(-MX) * negsum) - (diag - MX)
    m = loss_i.mean()
    if np.isnan(m):
        m = 0.0
    return np.float32(m)


def run(inputs, trace=False, **spmd_kwargs):
    from concourse.bass_utils import run_bass_kernel_spmd
    nc = _get_module()
    in_maps = _make_in_maps(inputs)
    res = run_bass_kernel_spmd(nc, in_maps, core_ids=list(range(NCORES)),
                               trace=trace, **spmd_kwargs)
    loss = _finalize(inputs, res.results)
    return loss, res


def kernel(**inputs) -> np.ndarray:
    loss, _ = run(inputs, trace=False)
    return loss
